# revision 25
# baseline (speedup 1.0000x reference)
"""DistWeightLoss Trainium2 kernel.

Problem: N=8192 embeddings of dim 128, K=8 instances per class (contiguous
blocks). loss = mean over rows of (mean of kept negative sims - sampled
positive sim + margin), where negatives are kept if sim > pos_min - margin.

Split of work:
  * O(N^2) work (the 8192x8192 similarity matrix + per-row thresholded
    sums/counts) runs on 8 NeuronCores, data-parallel over row slabs of
    1024 rows. Each core gets the full X^T (all-gather done host-side by
    replicating the input), computes its [1024, 8192] slab of sim with
    float32r matmuls (full PE rate, ~1e-4 rel precision), and reduces
    relu(sim - thr) and count(sim > thr) per row with fused accumulate ops:
    ACT does relu+bias+row-accum out of PSUM, DVE does is_gt+row-accum out
    of PSUM in parallel; a couple of groups use ACT Sign+accum instead of
    DVE to balance the two engines.
  * O(N) work (per-row positives from the 8x8 block-diagonal, sort,
    categorical sampling, threshold, same-class correction, final scalar)
    runs on host: ~17 MFLOP vs 17 GFLOP on device.

The device returns, per row, sum(relu(sim - thr)) and count(sim > thr) over
ALL columns; host subtracts the same-class (block) contribution computed
from host-side block sims, then loss_i = usum_neg/cnt_neg (cnt>0). Note
sum(relu(sim-thr)) over kept negatives == sum(sim*keep) - thr*cnt, so
loss_i = neg_mean - pos_min + margin exactly.
"""

import numpy as np

N = 8192
D = 128
K = 8
MARGIN = 0.01
NCORES = 8
ROWS = N // NCORES          # 1024 rows per core
RCH = ROWS // 128           # 8 row chunks of 128
CG = 2048                   # column group processed per fused op
NCG = N // CG               # 4 column groups
NMM = CG // 512             # 4 matmuls per group
NSTAT = RCH * NCG           # 32 partial-stat columns

# (r, g) groups whose count comes from ACT Sign (cnt = (acc + CG) / 2)
SIGN_GROUPS = {(3, 3)}
WARMUP_MMS = 12             # dummy f32r matmuls (~427ns each cold) to trip the
                            # PE HAM un-throttle during the DMA lead-in

_compiled = None            # built Bass module memo
last_results = None         # BassKernelResults of the most recent run (for test.py)


def _make_tile_context_cls():
    """TileContext subclass that splits multi-sem-wait instructions.

    The walrus in this container rejects instructions carrying more than one
    sync wait ("Too many sync wait commands", CoreV3GenImpl.cpp:104) — seen
    on both CTRL (Drain) and S3_LW (Matmult) structs. TileContext emits
    instructions waiting on several semaphores at once. Fix: before any
    instruction with >1 wait, insert same-engine EventSemaphore no-ops each
    absorbing one wait; engines execute in program order, so semantics are
    preserved.
    """
    from concourse import mybir
    import concourse.tile as tile

    class SplitWaitTileContext(tile.TileContext):
        MAX_WAITS = 1

        def _drain_and_barrier(self, tick_clock, wait_clock):
            super()._drain_and_barrier(tick_clock, wait_clock)
            self._split_wide_waits()

        def _split_wide_waits(self):
            nc = self.nc
            for bb in nc.main_func.blocks:
                insts = bb.instructions
                i = 0
                while i < len(insts):
                    insn = insts[i]
                    si = getattr(insn, "sync_info", None)
                    if si is not None and si.on_wait and len(si.on_wait) > self.MAX_WAITS:
                        waits = list(si.on_wait)
                        extra = waits[: -self.MAX_WAITS]
                        keep = waits[-self.MAX_WAITS :]
                        new_insts = []
                        for w in extra:
                            d = mybir.InstEventSemaphore(
                                name=nc.get_next_instruction_name(),
                                opcode="EventSemaphore",
                                engine=insn.engine,
                                ins=[],
                                outs=[],
                                sync_info=mybir.SyncInfo(on_wait=[w], on_update=[]),
                            )
                            nc.register_instruction(d, overwrite=True)
                            new_insts.append(d)
                        insn.sync_info = mybir.SyncInfo(
                            on_wait=keep, on_update=list(si.on_update)
                        )
                        for k, d in enumerate(new_insts):
                            insts.insert(i + k, d)
                        i += len(new_insts)
                    i += 1

    return SplitWaitTileContext


def _build_bass():
    import concourse.bass as bass
    from concourse import mybir

    SplitDrainTileContext = _make_tile_context_cls()

    f32 = mybir.dt.float32
    f32r = mybir.dt.float32r
    bf16 = mybir.dt.bfloat16

    nc = bass.Bass("TRN2", target_bir_lowering=False, debug=False)
    xT = nc.dram_tensor("xT", [128, N], f32r, kind="ExternalInput").ap()
    # negthr[p, r] = -(thr of row r*128+p of this core's slab)
    negthr = nc.dram_tensor("negthr", [128, RCH], f32, kind="ExternalInput").ap()
    # core_col0: first global column of this core's slab inside xT; passed as
    # a compile-time-constant per-core offset is not possible in SPMD, so the
    # slab's lhsT slice is taken from the xt tile holding those columns.
    us_out = nc.dram_tensor("us_out", [128, NSTAT], f32, kind="ExternalOutput").ap()
    cnt_out = nc.dram_tensor("cnt_out", [128, NSTAT], f32, kind="ExternalOutput").ap()
    # per-core slab lhsT, [128 d, 1024 rows] — a column slice of xT; passed
    # separately so the same SPMD program works on every core.
    xsT = nc.dram_tensor("xsT", [128, ROWS], f32r, kind="ExternalInput").ap()

    with SplitDrainTileContext(nc) as tc:
        with (
            tc.tile_pool(name="persist", bufs=1) as persist,
            tc.tile_pool(name="psum", bufs=2, space="PSUM") as psum,
            tc.tile_pool(name="relu", bufs=6) as relu_pool,
            tc.tile_pool(name="junk", bufs=3) as junk_pool,
        ):
            # DMA order matters: the first matmul group needs only the first
            # row-chunk's lhsT (64KB) + the first 2048 columns of xT (1MiB);
            # everything else streams in behind while compute runs (g-major
            # loop order keeps each xt tile busy for ~17us of compute).
            xs0_sb = persist.tile([128, 128], f32r, tag="xs0")
            nc.sync.dma_start(xs0_sb[:], xsT[:, 0:128])
            xt_sb = []
            t = persist.tile([128, CG], f32r, tag="xt0")
            nc.sync.dma_start(t[:], xT[:, 0:CG])
            xt_sb.append(t)
            nthr_sb = persist.tile([128, RCH], f32, tag="nthr")
            nc.sync.dma_start(nthr_sb[:], negthr[:])
            xs_sb = persist.tile([128, ROWS - 128], f32r, tag="xs")
            nc.sync.dma_start(xs_sb[:], xsT[:, 128:ROWS])
            for g in range(1, NCG):
                t = persist.tile([128, CG], f32r, tag=f"xt{g}")
                nc.sync.dma_start(t[:], xT[:, g * CG : (g + 1) * CG])
                xt_sb.append(t)
            us_stats = persist.tile([128, NSTAT], f32, tag="us_stats")
            cnt_stats = persist.tile([128, NSTAT], f32, tag="cnt_stats")

            # PE warmup: dense dummy matmuls during the DMA lead-in trip the
            # HAM clock gate to 8/8; the real matmul bursts then never idle
            # long enough (<~5us) to re-throttle, halving per-MM time.
            dummy = persist.tile([128, 512], f32, tag="dummy")
            nc.gpsimd.memset(dummy[:], 0.0)
            # Prefetch the ACT table set during the DMA lead-in: walrus puts
            # the ~2.7us ACT_TABLE_LOAD before the first ACTIVATE in the ACT
            # stream; give it a dep-free dummy so it doesn't gate group 0.
            dumact = persist.tile([128, 1], f32, tag="dumact")
            nc.scalar.activation(
                dumact[:], dummy[:, 0:1], mybir.ActivationFunctionType.Relu
            )
            wps = psum.tile([128, CG], f32, tag="ps")
            for w in range(WARMUP_MMS):
                nc.tensor.matmul(
                    wps[:, (w % NMM) * 512 : (w % NMM + 1) * 512],
                    lhsT=dummy[:, 0:128].bitcast(f32r),
                    rhs=dummy[:].bitcast(f32r),
                    start=True,
                    stop=True,
                )

            for g in range(NCG):
                for r in range(RCH):
                    lhs = (
                        xs0_sb[:]
                        if r == 0
                        else xs_sb[:, (r - 1) * 128 : r * 128]
                    )
                    ps = psum.tile([128, CG], f32, tag="ps")
                    for k in range(NMM):
                        nc.tensor.matmul(
                            ps[:, k * 512 : (k + 1) * 512],
                            lhsT=lhs,
                            rhs=xt_sb[g][:, k * 512 : (k + 1) * 512],
                            start=True,
                            stop=True,
                        )
                    idx = r * NCG + g
                    rl = relu_pool.tile([128, CG], bf16)
                    nc.scalar.activation(
                        rl[:],
                        ps[:],
                        mybir.ActivationFunctionType.Relu,
                        bias=nthr_sb[:, r : r + 1],
                        scale=1.0,
                        accum_out=us_stats[:, idx : idx + 1],
                    )
                    if (r, g) in SIGN_GROUPS:
                        sj = junk_pool.tile([128, CG], bf16)
                        nc.scalar.activation(
                            sj[:],
                            ps[:],
                            mybir.ActivationFunctionType.Sign,
                            bias=nthr_sb[:, r : r + 1],
                            scale=1.0,
                            accum_out=cnt_stats[:, idx : idx + 1],
                        )
                    else:
                        junk = junk_pool.tile([128, CG], bf16)
                        nc.vector.tensor_scalar(
                            out=junk[:],
                            in0=rl[:],
                            scalar1=0.0,
                            scalar2=None,
                            op0=mybir.AluOpType.is_gt,
                            op1=mybir.AluOpType.add,
                            accum_out=cnt_stats[:, idx : idx + 1],
                        )

            nc.sync.dma_start(us_out[:], us_stats[:])
            nc.sync.dma_start(cnt_out[:], cnt_stats[:])

    return nc


def _get_compiled():
    global _compiled
    if _compiled is None:
        _compiled = _build_bass()
    return _compiled


def _host_phase1(X):
    """Per-row threshold thr = pos_min - margin, plus block sims for the
    same-class correction. All O(N*K*D)."""
    import jax
    import jax.numpy as jnp

    X3 = X.reshape(N // K, K, D)
    B = np.einsum("cid,cjd->cij", X3, X3)          # [N/K, K, K] block sims
    ci = np.arange(N) // K
    ji = np.arange(N) % K
    ball = B[ci, ji, :]                             # [N, K] same-class sims (incl diag)
    off = (ji[:, None] + 1 + np.arange(K - 1)[None, :]) % K
    pos = ball[np.arange(N)[:, None], off]          # [N, K-1]
    pos_sorted = np.sort(pos, axis=1)
    samp = np.asarray(
        jax.random.categorical(
            jax.random.key(42), 5.0 * jnp.asarray(pos_sorted), axis=-1
        )
    )
    pos_min = pos_sorted[np.arange(N), samp]
    thr = (pos_min - MARGIN).astype(np.float32)
    return thr, ball


def kernel(inputs: np.ndarray, targets: np.ndarray) -> np.ndarray:
    from concourse.bass_utils import run_bass_kernel_spmd

    X = np.ascontiguousarray(np.asarray(inputs, dtype=np.float32))
    assert X.shape == (N, D)

    thr, ball = _host_phase1(X)

    XT = np.ascontiguousarray(X.T)                  # [128, 8192]
    in_maps = []
    for m in range(NCORES):
        slab_thr = thr[m * ROWS : (m + 1) * ROWS].reshape(RCH, 128).T
        in_maps.append(
            {
                "xT": XT,
                "xsT": np.ascontiguousarray(XT[:, m * ROWS : (m + 1) * ROWS]),
                "negthr": np.ascontiguousarray(-slab_thr),
            }
        )

    nc = _get_compiled()
    res = run_bass_kernel_spmd(nc, in_maps, list(range(NCORES)))
    global last_results
    last_results = res

    sign_mask = np.zeros((RCH, NCG), dtype=bool)
    for (r, g) in SIGN_GROUPS:
        sign_mask[r, g] = True

    usum = np.empty(N, dtype=np.float64)
    cnt = np.empty(N, dtype=np.float64)
    for m in range(NCORES):
        us = res.results[m]["us_out"].reshape(128, RCH, NCG).astype(np.float64)
        cn = res.results[m]["cnt_out"].reshape(128, RCH, NCG).astype(np.float64)
        # sign groups: acc = cnt_gt - cnt_lt, with cnt_gt + cnt_lt = CG (ties
        # have measure zero) => cnt_gt = (acc + CG) / 2
        cn = np.where(sign_mask[None, :, :], (cn + CG) / 2.0, cn)
        usum[m * ROWS : (m + 1) * ROWS] = us.sum(axis=2).T.reshape(ROWS)
        cnt[m * ROWS : (m + 1) * ROWS] = cn.sum(axis=2).T.reshape(ROWS)

    # subtract same-class (block incl diagonal) contributions, host-side
    t = ball.astype(np.float64) - thr[:, None].astype(np.float64)
    corr_us = np.maximum(t, 0.0).sum(axis=1)
    corr_cnt = (t > 0.0).sum(axis=1)
    us_neg = usum - corr_us
    cnt_neg = np.rint(cnt - corr_cnt)
    loss_i = np.where(cnt_neg > 0.5, us_neg / np.maximum(cnt_neg, 1.0), 0.0)
    loss = loss_i.sum() / N
    return np.float32(loss)
